# revision 10
# baseline (speedup 1.0000x reference)
"""Trainium2 Bass kernel for MHA with ALiBi + causal mask.

Problem: B=2, S=2048, D_MODEL=2048, H=16, HEAD_DIM=128, fp32 I/O.
Sharding: tensor-parallel over heads — core c owns heads [2c, 2c+2) for both
batches. Each core computes its heads' Q/K/V projections, attention, and a
rank-256 partial of the output projection; a ReduceScatter sums the partials
on device and each core returns a disjoint fp16 slice of the final output.

The run is transfer-bound over the axon tunnel (~40 MiB/s, serialized), so
the layout minimizes host<->device bytes:
  - x is uploaded sharded (512 rows fp16 per core) and AllGathered on device
    into a Shared DRAM buffer
  - alibi is causally packed (only k-tiles at or below the diagonal per
    512-wide q chunk) and shipped as fp8 e4m3; mask fill is -224 so
    exp(score + bias) underflows to 0
  - output partials are ReduceScattered on device; each core emits its
    512-row slice as fp16 (host converts to f32 and concatenates)

Device pipeline per core:
  phase 0: AllGather x shards -> Shared DRAM
  phase 1: Q^T,K^T (weights stationary) and V natural (x stationary), fp16
  phase 2: scores^T = K @ Q^T per 128x512 block, packed fp8 ALiBi upcast to
           fp16 and added on VectorE, exp on ScalarE, softmax denominators
           via ones-vector matmul, PV accumulation (out^T layout), normalize
           fused into PSUM eviction (reciprocal broadcast via matmul)
  phase 3: partial output projection, f32 into DRAM scratch
  phase 4: ReduceScatter(add) partials, cast own slice to fp16, emit
"""

import numpy as np
import ml_dtypes

D_MODEL = 2048
N_HEADS = 16
HEAD_DIM = 128
BATCH = 2
SEQ = 2048
N_CORES = 8
H_LOC = 2          # heads per core
EC = 16            # 128-row chunks of the d_model contraction dim
SC = 512           # s-chunk (matmul free dim)
BS = BATCH * SEQ   # 4096
NEG = -224.0       # causal mask fill, fp8 e4m3-representable, exp -> 0
FBASE = (0, 0, 4, 12)   # full-tile base per q-chunk (tiles strictly below diag)
DPAIRS = tuple((kt, qt) for kt in range(4) for qt in range(kt, 4))  # 10 pairs

_cache = {}


def _build():
    import concourse.mybir as mybir
    from concourse import bacc
    import concourse.tile as tile
    from concourse.masks import make_identity

    FP16 = mybir.dt.float16
    FP8 = mybir.dt.float8e4
    F32 = mybir.dt.float32
    P = 128

    nc = bacc.Bacc(None, target_bir_lowering=False, num_devices=N_CORES)

    xs_d = nc.dram_tensor("xs", [P, EC, SC], FP16, kind="ExternalInput")
    wq_d = nc.dram_tensor("wqT", [P, EC, H_LOC * HEAD_DIM], FP16, kind="ExternalInput")
    wk_d = nc.dram_tensor("wkT", [P, EC, H_LOC * HEAD_DIM], FP16, kind="ExternalInput")
    wv_d = nc.dram_tensor("wvT", [P, EC, H_LOC * HEAD_DIM], FP16, kind="ExternalInput")
    wo_d = nc.dram_tensor("woT", [P, H_LOC, D_MODEL], FP16, kind="ExternalInput")
    alf_d = nc.dram_tensor("alibiF", [H_LOC, P, 24, SC], FP8, kind="ExternalInput")
    ald_d = nc.dram_tensor("alibiD", [H_LOC, P, 40, P], FP8, kind="ExternalInput")
    y_d = nc.dram_tensor("y16", [BS // P // N_CORES, P, D_MODEL], FP16,
                         kind="ExternalOutput")

    mult = mybir.AluOpType.mult
    Exp = mybir.ActivationFunctionType.Exp

    with tile.TileContext(nc) as tc:
        with tc.tile_pool(name="dram", bufs=1, space="DRAM") as dpool, \
             tc.tile_pool(name="const", bufs=1) as constp, \
             tc.tile_pool(name="wpool", bufs=1) as wpool, \
             tc.tile_pool(name="qkv", bufs=1) as qkvp, \
             tc.tile_pool(name="xp", bufs=2) as xp, \
             tc.tile_pool(name="attn", bufs=4) as apool, \
             tc.tile_pool(name="ali8", bufs=2) as bpool8, \
             tc.tile_pool(name="ali", bufs=2) as bpool, \
             tc.tile_pool(name="rcp", bufs=4) as rcpool, \
             tc.tile_pool(name="rbp", bufs=2) as rbpool, \
             tc.tile_pool(name="yp", bufs=4) as ypool:

            # ---- phase 0: AllGather x shards into a Shared DRAM buffer ----
            x_in = dpool.tile([P, EC, SC], FP16, name="x_in")
            xg = dpool.tile([N_CORES, P, EC, SC], FP16, addr_space="Shared",
                            name="xg")
            nc.gpsimd.dma_start(x_in[:], xs_d[:])
            nc.gpsimd.collective_compute(
                "AllGather", mybir.AluOpType.bypass,
                replica_groups=[list(range(N_CORES))],
                ins=[x_in.opt()], outs=[xg.opt()])

            yp_dram = dpool.tile([BS // P, P, D_MODEL], F32, name="yp_dram")
            yr_dram = dpool.tile([BS // P // N_CORES, P, D_MODEL], F32,
                                 name="yr_dram")

            ident = constp.tile([P, P], FP16, tag="ident", name="ident")
            make_identity(nc, ident)
            ones = constp.tile([P, 1], FP16, tag="ones", name="ones")
            nc.vector.memset(ones, 1.0)
            ones1 = constp.tile([1, P], F32, tag="ones1", name="ones1")
            nc.vector.memset(ones1, 1.0)

            wq = wpool.tile([P, EC, 256], FP16, tag="wq", name="wq")
            wk = wpool.tile([P, EC, 256], FP16, tag="wk", name="wk")
            wv = wpool.tile([P, EC, 256], FP16, tag="wv", name="wv")
            wo = wpool.tile([P, H_LOC, D_MODEL], FP16, tag="wo", name="wo")
            nc.sync.dma_start(out=wq, in_=wq_d[:, :, :])
            nc.sync.dma_start(out=wk, in_=wk_d[:, :, :])
            nc.sync.dma_start(out=wv, in_=wv_d[:, :, :])
            nc.sync.dma_start(out=wo, in_=wo_d[:, :, :])

            # persistent per-(batch, head) activations, fp16
            QT = [[qkvp.tile([P, SEQ], FP16, tag=f"q{b}{h}", name=f"q{b}{h}") for h in range(2)]
                  for b in range(2)]
            KT = [[qkvp.tile([P, SEQ], FP16, tag=f"k{b}{h}", name=f"k{b}{h}") for h in range(2)]
                  for b in range(2)]
            V = [qkvp.tile([P, EC, 256], FP16, tag=f"v{b}", name=f"v{b}") for b in range(2)]
            OT = [[qkvp.tile([P, SEQ], FP16, tag=f"o{b}{h}", name=f"o{b}{h}") for h in range(2)]
                  for b in range(2)]

            # ---- phase 1: projections ----
            with tc.tile_pool(name="ps1", bufs=4, space="PSUM") as ps_qk, \
                 tc.tile_pool(name="ps1v", bufs=3, space="PSUM") as ps_v:
                for c8 in range(BS // SC):          # 8 chunks of 512 rows of x
                    b, scn = c8 // 4, c8 % 4
                    xt = xp.tile([P, EC, SC], FP16, tag="xt", name="xt")
                    nc.sync.dma_start(out=xt, in_=xg[c8])
                    for W_sb, dest in ((wq, QT), (wk, KT)):
                        for h in range(2):
                            ps = ps_qk.tile([P, SC], F32, tag="qk", name="qk")
                            for e in range(EC):
                                nc.tensor.matmul(
                                    ps,
                                    lhsT=W_sb[:, e, h * P:(h + 1) * P],
                                    rhs=xt[:, e, :],
                                    start=(e == 0), stop=(e == EC - 1))
                            nc.scalar.copy(
                                out=dest[b][h][:, scn * SC:(scn + 1) * SC], in_=ps)
                    for st in range(SC // P):       # V natural, 4 tiles of 128
                        psv = ps_v.tile([P, 256], F32, tag="v")
                        for e in range(EC):
                            nc.tensor.matmul(
                                psv,
                                lhsT=xt[:, e, st * P:(st + 1) * P],
                                rhs=wv[:, e, :],
                                start=(e == 0), stop=(e == EC - 1))
                        tv = scn * 4 + st
                        nc.scalar.copy(out=V[b][:, tv, :], in_=psv)

            # ---- phase 2: attention ----
            with tc.tile_pool(name="ps2s", bufs=3, space="PSUM") as ps_sc, \
                 tc.tile_pool(name="ps2o", bufs=2, space="PSUM") as ps_out, \
                 tc.tile_pool(name="ps2m", bufs=2, space="PSUM") as ps_sum, \
                 tc.tile_pool(name="ps2b", bufs=1, space="PSUM") as ps_bc:
                for h in range(2):
                    for qj in range(SEQ // SC):     # 4 query chunks of 512
                        nkt = 4 * qj + 4            # causal: k tiles 0..4qj+3
                        out_ps = [ps_out.tile([P, SC], F32, tag="out", name="out")
                                  for _ in range(2)]
                        sum_ps = [ps_sum.tile([1, SC], F32, tag="sum", name="sum")
                                  for _ in range(2)]
                        for ki in range(nkt):
                            if ki % 4 == 0:
                                a8 = bpool8.tile([P, 4, SC], FP8, tag="ali8",
                                                 name="ali8")
                                if ki < 4 * qj:     # full below-diagonal group
                                    nc.sync.dma_start(
                                        out=a8,
                                        in_=alf_d[h, :, FBASE[qj] + ki:
                                                  FBASE[qj] + ki + 4, :])
                                else:               # diagonal-crossing square
                                    for kt in range(4):
                                        nc.vector.memset(a8[:, kt, :], NEG)
                                    for i, (kt, qt) in enumerate(DPAIRS):
                                        nc.sync.dma_start(
                                            out=a8[:, kt,
                                                   qt * P:(qt + 1) * P],
                                            in_=ald_d[h, :, 10 * qj + i, :])
                                a = bpool.tile([P, 4, SC], FP16, tag="ali",
                                               name="ali")
                                nc.scalar.copy(out=a, in_=a8)
                            for b in range(2):
                                sc_ps = ps_sc.tile([P, SC], F32, tag="sc", name="sc")
                                nc.tensor.matmul(
                                    sc_ps,
                                    lhsT=KT[b][h][:, ki * P:(ki + 1) * P],
                                    rhs=QT[b][h][:, qj * SC:(qj + 1) * SC],
                                    start=True, stop=True)
                                at32 = apool.tile([P, SC], F32, tag="at32",
                                                  name="at32")
                                nc.vector.scalar_tensor_tensor(
                                    out=at32, in0=sc_ps, scalar=1.0,
                                    in1=a[:, ki % 4, :],
                                    op0=mult, op1=mybir.AluOpType.add)
                                at = apool.tile([P, SC], FP16, tag="at", name="at")
                                nc.scalar.activation(at, at32, Exp)
                                nc.tensor.matmul(sum_ps[b], lhsT=ones, rhs=at,
                                                 start=(ki == 0),
                                                 stop=(ki == nkt - 1))
                                nc.tensor.matmul(
                                    out_ps[b],
                                    lhsT=V[b][:, ki, h * P:(h + 1) * P],
                                    rhs=at,
                                    start=(ki == 0), stop=(ki == nkt - 1))
                        for b in range(2):
                            rc = rcpool.tile([1, SC], F32, tag="rc", name="rc")
                            nc.vector.reciprocal(out=rc, in_=sum_ps[b])
                            bc = ps_bc.tile([P, SC], F32, tag="bc", name="bc")
                            nc.tensor.matmul(bc, lhsT=ones1, rhs=rc,
                                             start=True, stop=True)
                            rb = rbpool.tile([P, SC], F32, tag="rb", name="rb")
                            nc.scalar.copy(out=rb, in_=bc)
                            nc.vector.scalar_tensor_tensor(
                                out=OT[b][h][:, qj * SC:(qj + 1) * SC],
                                in0=out_ps[b], scalar=1.0, in1=rb,
                                op0=mult, op1=mult)

            # ---- phase 3: output projection (rank-256 partial) ----
            with tc.tile_pool(name="ps3", bufs=4, space="PSUM") as ps_y:
                for b in range(2):
                    for st in range(SEQ // P):      # 16 row tiles per batch
                        ysb = ypool.tile([P, D_MODEL], F32, tag="ysb",
                                         name="ysb")
                        for mj in range(D_MODEL // SC):
                            yp = ps_y.tile([P, SC], F32, tag="y", name="y")
                            for h in range(2):
                                nc.tensor.matmul(
                                    yp,
                                    lhsT=OT[b][h][:, st * P:(st + 1) * P],
                                    rhs=wo[:, h, mj * SC:(mj + 1) * SC],
                                    start=(h == 0), stop=(h == 1))
                            if mj % 2 == 0:
                                nc.scalar.copy(
                                    out=ysb[:, mj * SC:(mj + 1) * SC], in_=yp)
                            else:
                                nc.vector.tensor_copy(
                                    out=ysb[:, mj * SC:(mj + 1) * SC], in_=yp)
                        nc.sync.dma_start(out=yp_dram[b * 16 + st], in_=ysb)

            # ---- phase 4: ReduceScatter partials, cast own slice to fp16 ----
            with tc.tile_pool(name="cast", bufs=2) as castp:
                nc.gpsimd.collective_compute(
                    "ReduceScatter", mybir.AluOpType.add,
                    replica_groups=[list(range(N_CORES))],
                    ins=[yp_dram.opt()], outs=[yr_dram.opt()])
                for j in range(BS // P // N_CORES):
                    t32 = ypool.tile([P, D_MODEL], F32, tag="ysb", name="t32")
                    nc.sync.dma_start(out=t32, in_=yr_dram[j])
                    t16 = castp.tile([P, D_MODEL], FP16, tag="t16", name="t16")
                    nc.scalar.copy(out=t16, in_=t32)
                    nc.sync.dma_start(out=y_d[j], in_=t16)
    nc.compile()
    return nc


def _prep_inputs(x, alibi_bias, W_q, W_k, W_v, W_o):
    f16 = np.float16
    f8 = ml_dtypes.float8_e4m3
    x = np.asarray(x, np.float32).reshape(BS, D_MODEL).astype(f16)
    # xT[p, e_chunk, s] with e = e_chunk*128 + p
    xT = np.ascontiguousarray(x.T.reshape(EC, 128, BS).transpose(1, 0, 2))

    scale = 1.0 / np.sqrt(np.float32(HEAD_DIM))
    neg8 = np.float32(NEG).astype(f8)
    tri = np.tril(np.ones((128, 128), dtype=bool), -1)  # [k,q] k>q strict

    in_maps = []
    for c in range(N_CORES):
        rows = slice(c * 256, (c + 1) * 256)

        def wt(W, s=1.0):
            # [e=2048, d_loc=256] -> [p, e_chunk, d]
            wT = (np.asarray(W, np.float32)[rows] * s).T
            return np.ascontiguousarray(
                wT.reshape(EC, 128, 256).transpose(1, 0, 2).astype(f16))

        woT = np.asarray(W_o, np.float32)[:, rows].T      # [256, 2048]
        woT = np.ascontiguousarray(
            woT.reshape(H_LOC, 128, D_MODEL).transpose(1, 0, 2).astype(f16))

        alfs, alds = [], []
        for hl in range(H_LOC):
            B8 = np.asarray(alibi_bias[2 * c + hl], np.float32).astype(f8)
            B5 = B8.reshape(4, SC, EC, 128)    # [qj, qcol, ktile, p]
            full = np.concatenate(
                [B5[qj, :, :4 * qj, :] for qj in range(1, 4)], axis=1)
            alfs.append(full.transpose(2, 1, 0))       # [p, 24, qcol]
            subs = []
            for qj in range(4):
                for kt, qt in DPAIRS:
                    k0 = (4 * qj + kt) * 128
                    q0 = qj * SC + qt * 128
                    sub = B8[q0:q0 + 128, k0:k0 + 128].T   # [k=p, q]
                    if kt == qt:
                        sub = np.where(tri, neg8, sub)
                    subs.append(sub)
            alds.append(np.stack(subs).transpose(1, 0, 2))  # [p, 40, q]
        alibiF = np.ascontiguousarray(np.stack(alfs))       # [hl,p,24,q]
        alibiD = np.ascontiguousarray(np.stack(alds))       # [hl,p,40,q]

        in_maps.append({
            "xs": np.ascontiguousarray(xT[:, :, c * SC:(c + 1) * SC]),
            "wqT": wt(W_q, scale),
            "wkT": wt(W_k),
            "wvT": wt(W_v),
            "woT": woT,
            "alibiF": alibiF,
            "alibiD": alibiD,
        })
    return in_maps


def kernel(x, alibi_bias, W_q, W_k, W_v, W_o, _trace=False):
    import time as _time
    from concourse.bass_utils import run_bass_kernel_spmd

    if "nc" not in _cache:
        _cache["nc"] = _build()
    nc = _cache["nc"]

    t0 = _time.time()
    in_maps = _prep_inputs(x, alibi_bias, W_q, W_k, W_v, W_o)
    _cache["prep_s"] = _time.time() - t0
    t0 = _time.time()
    res = run_bass_kernel_spmd(nc, in_maps, core_ids=list(range(N_CORES)),
                               trace=_trace)
    _cache["run_s"] = _time.time() - t0
    _cache["last_result"] = res
    shards = [np.asarray(om["y16"]) for om in res.results]
    y = np.concatenate(shards, axis=0).astype(np.float32)
    return y.reshape(BATCH, SEQ, D_MODEL)
